# revision 7
# baseline (speedup 1.0000x reference)
"""Trainium2 Bass kernel for CFContrastiveLoss.

Reference semantics (per sample of N=16 options, D=768 dims):
  - L2-normalize option embeddings
  - sim = pairwise cosine sims within the sample (16x16 gram)
  - max_neg[n] = max over negative-labeled columns of sim[n, :]
  - loss = mean over (positive rows of valid samples) of relu(max_neg + 0.3)

Device strategy (pure data parallel over batch, 8 cores):
  - 128 rows (= 8 samples x 16 options) per "group"; per core 16384 rows
    = 128 groups.
  - Host pre-normalizes embeddings, splits each fp32 value into a bf16
    (hi, lo) pair (hi = bf16(x), lo = bf16(x - hi)) and pre-transposes to
    the matmul layout.  The gram matrix is computed on the TensorEngine as
    Hi.T@Hi + Hi.T@Lo + Lo.T@Hi (the Lo.T@Lo term is ~1e-7 and dropped),
    giving fp32-grade accuracy at bf16 matmul speed (fp32 matmuls are 4x
    slower on TRN2).  DMA volume is identical to fp32 (2x2B per element).
  - The label/validity masking is folded into the same PSUM accumulation
    as one extra K=10 matmul of +-2^30 sentinel outer products:
      row 0:  ones x negc           (negc[m] = -2^30 iff label[m] == 1)
      row 1:  ones x (-2^30 * ones) (mask everything ...)
      row 2+s: u_s x (+2^30 * u_s)  (... except within-sample blocks)
    Sentinels are powers of two, so in-block negative columns get an
    exactly-zero mask contribution and unmasked sims are bit-exact.
  - Per group the device then does a single VectorE row-max from PSUM.
    relu/margin/weighting/final mean are O(rows) and done on host.
"""

import os

import numpy as np
import ml_dtypes

import concourse.bass as bass
import concourse.mybir as mybir
from concourse import bacc, tile
from concourse.bass_utils import run_bass_kernel_spmd

BF16 = mybir.dt.bfloat16
F32 = mybir.dt.float32

B, N, D = 8192, 16, 768
N_CORES = 8
ROWS = B * N                      # 131072
ROWS_PER_CORE = ROWS // N_CORES   # 16384
GROUPS = ROWS_PER_CORE // 128     # 128 groups of 128 rows per core
KCH = D // 128                    # 6 contraction chunks
BATCH = 16                        # groups per DMA'd output tile
NBATCH = GROUPS // BATCH          # 8
MASK_K = 2 + 128 // N             # 10 mask matmul rows
SENT = np.float32(2.0 ** 30)
MARGIN = np.float32(0.3)

_CACHE: dict = {}

LAST_RESULT = None  # BassKernelResults of the most recent device run


def _build_program() -> bass.Bass:
    nc = bacc.Bacc(None)
    ethi = nc.declare_dram_parameter("ethi", [GROUPS, 128, D], BF16, isOutput=False)
    etlo = nc.declare_dram_parameter("etlo", [GROUPS, 128, D], BF16, isOutput=False)
    mrhs = nc.declare_dram_parameter("mrhs", [GROUPS, MASK_K, 128], BF16, isOutput=False)
    mlhs = nc.declare_dram_parameter("mlhs", [MASK_K, 128], BF16, isOutput=False)
    out = nc.declare_dram_parameter("out", [NBATCH, 128, BATCH], F32, isOutput=True)

    with tile.TileContext(nc) as tc:
        with (
            tc.tile_pool(name="emb", bufs=4) as emb_pool,
            tc.tile_pool(name="msk", bufs=4) as msk_pool,
            tc.tile_pool(name="const", bufs=1) as const_pool,
            tc.tile_pool(name="wide", bufs=2) as wide_pool,
            tc.tile_pool(name="psum", bufs=8, space="PSUM") as psum_pool,
        ):
            mlhs_t = const_pool.tile([MASK_K, 128], BF16)
            nc.gpsimd.dma_start(mlhs_t[:], mlhs[:])

            for b in range(NBATCH):
                wide = wide_pool.tile([128, BATCH], F32)
                for j in range(BATCH):
                    g = b * BATCH + j
                    hi = emb_pool.tile([128, D], BF16, tag="hi")
                    lo = emb_pool.tile([128, D], BF16, tag="lo")
                    mr = msk_pool.tile([MASK_K, 128], BF16, tag="mr")
                    nc.gpsimd.dma_start(hi[:], ethi[g])
                    nc.gpsimd.dma_start(lo[:], etlo[g])
                    nc.gpsimd.dma_start(mr[:], mrhs[g])

                    ps = psum_pool.tile([128, 512], F32)  # one full PSUM bank
                    G = ps[:, 0:128]
                    # Mask sentinels first (start=True clears the bank).
                    nc.tensor.matmul(G, mlhs_t[:], mr[:], start=True, stop=False)
                    for k in range(KCH):
                        hk = hi[:, k * 128:(k + 1) * 128]
                        lk = lo[:, k * 128:(k + 1) * 128]
                        nc.tensor.matmul(G, hk, hk, start=False, stop=False)
                        nc.tensor.matmul(G, hk, lk, start=False, stop=False)
                        nc.tensor.matmul(G, lk, hk, start=False, stop=(k == KCH - 1))
                    nc.vector.reduce_max(wide[:, j:j + 1], G, axis=mybir.AxisListType.X)
                nc.gpsimd.dma_start(out[b], wide[:])
    nc.finalize()
    return nc


def _prep_core_inputs(Xn: np.ndarray, lab: np.ndarray):
    """Per-core input maps from normalized embeddings + flat labels."""
    hi = Xn.astype(ml_dtypes.bfloat16)
    lo = (Xn - hi.astype(np.float32)).astype(ml_dtypes.bfloat16)

    negc = np.where(lab == 1, -SENT, np.float32(0.0)).astype(np.float32)

    m_idx = np.arange(128)
    # mask lhsT: [10, 128] = [ones; ones; u_0..u_7]
    mlhs = np.zeros((MASK_K, 128), dtype=np.float32)
    mlhs[0, :] = 1.0
    mlhs[1, :] = 1.0
    for s in range(128 // N):
        mlhs[2 + s, :] = (m_idx // N == s).astype(np.float32)
    mlhs_bf = mlhs.astype(ml_dtypes.bfloat16)

    # static part of mask rhs rows 1..9
    mrhs_static = np.zeros((MASK_K, 128), dtype=np.float32)
    mrhs_static[1, :] = -SENT
    for s in range(128 // N):
        mrhs_static[2 + s, :] = np.where(m_idx // N == s, SENT, np.float32(0.0))

    in_maps = []
    for c in range(N_CORES):
        r0 = c * ROWS_PER_CORE
        r1 = r0 + ROWS_PER_CORE
        hi_c = hi[r0:r1]
        lo_c = lo[r0:r1]
        # [rows=16384, 768] -> [g, n, k, p] -> [g, p, k, n] (p = dim-in-chunk)
        ethi = np.ascontiguousarray(
            hi_c.reshape(GROUPS, 128, KCH, 128).transpose(0, 3, 2, 1)
        ).reshape(GROUPS, 128, D)
        etlo = np.ascontiguousarray(
            lo_c.reshape(GROUPS, 128, KCH, 128).transpose(0, 3, 2, 1)
        ).reshape(GROUPS, 128, D)
        mrhs = np.broadcast_to(mrhs_static, (GROUPS, MASK_K, 128)).copy()
        mrhs[:, 0, :] = negc[r0:r1].reshape(GROUPS, 128)
        in_maps.append({
            "ethi": ethi,
            "etlo": etlo,
            "mrhs": mrhs.astype(ml_dtypes.bfloat16),
            "mlhs": mlhs_bf,
        })
    return in_maps


def kernel(embeddings: np.ndarray, labels: np.ndarray) -> np.ndarray:
    global LAST_RESULT
    assert embeddings.shape == (B, N, D)
    assert labels.shape == (B, N)

    X = np.asarray(embeddings, dtype=np.float32).reshape(ROWS, D)
    lab = np.asarray(labels).reshape(ROWS)

    ss = np.square(X).sum(axis=1, dtype=np.float32)
    norms = np.sqrt(ss)
    Xn = X / np.maximum(norms, np.float32(1e-12))[:, None]

    in_maps = _prep_core_inputs(Xn, lab)

    if "nc" not in _CACHE:
        _CACHE["nc"] = _build_program()
    nc = _CACHE["nc"]

    trace = os.environ.get("BASS_KERNEL_TRACE", "0") == "1"
    res = run_bass_kernel_spmd(nc, in_maps, list(range(N_CORES)), trace=trace)
    LAST_RESULT = res

    # out[b, p, j]: group g = b*BATCH+j, row-within-group p
    maxneg = np.concatenate(
        [np.asarray(r["out"]).transpose(0, 2, 1).reshape(ROWS_PER_CORE)
         for r in res.results]
    )

    triplet = np.maximum(maxneg + MARGIN, np.float32(0.0))
    has_neg = (np.asarray(labels) == 0).any(axis=1)
    w = (lab == 1) & np.repeat(has_neg, N)
    loss_sum = np.float32((triplet * w).sum(dtype=np.float64))
    count = int(w.sum())
    loss = np.float32(loss_sum / np.float32(max(count, 1)))
    return np.asarray(loss, dtype=np.float32)
